# revision 9
# baseline (speedup 1.0000x reference)
"""ARMA filter on 8 NeuronCores via Bass/Tile.

y_t = tanh(ma_t + tanh(arnet(y_{t-8..t-1}))), ma = tanh(causal 9-tap conv(x)).

Sharding: data-parallel over batch (64 rows -> 8 cores x 8 rows). Identical
SPMD program on every core; no collectives.

Per-core layout ("transposed"): feature dim on partitions. Per-step tensors
are [128 partitions (f half), 16 free (f-chunk m in {0,1} x batch b in 0..8)].
  - MA branch: 18 accumulating GEMMs (K = 9 taps x 256 feat) per output tile,
    N=512 time-columns per matmul; tanh via ScalarE into a per-step
    interleaved SBUF buffer.
  - AR branch: tap with lag d can be batched over at most d consecutive
    steps. Lag 1 fires every step (the serial chain); lags 2,3 fire every 2
    steps; lags 4..7 every 4; lag 8 every 8. All taps accumulate into one
    PSUM tile per 8-step block (cols = step*16 + m*8 + b), initialized by a
    K=1 matmul that adds b_ar.
  - Serial chain per step: PE (4 matmuls) -> ACT tanh -> DVE add ma -> ACT
    tanh -> y to SBUF (bf16) feeding the next step's matmuls.

All matmul data is bf16 (fp32 PSUM accumulation); tolerance is 2e-2.
"""

import os

import numpy as np
import ml_dtypes

F = 256
MASZ = 8  # MA taps minus current frame (conv length 9)
ARSZ = 8
B_TOT = 64
NCORES = 8
BL = B_TOT // NCORES  # 8 batch rows per core
T_FULL = 2048
TT = 512  # MA time tile

_CACHE = {}


def _bf16(a):
    return np.asarray(a, dtype=np.float32).astype(ml_dtypes.bfloat16)


def _patch_tile_drain():
    """Split the Tile tail-drain's sem waits across single-wait NOPs.

    The nix walrus in this container rejects instructions with >2 sync
    waits ("Too many sync wait commands"), and TileContext's exit drain
    waits on the whole vector clock.
    """
    import concourse.tile as tile
    import concourse.mybir as mybir
    from concourse.vector_clock import ScopedClock

    if getattr(tile.TileContext, "_arma_drain_patched", False):
        return

    def _drain_and_barrier(self, tick_clock, wait_clock):
        nc = self.nc
        holder = nc.sync.nop(nofuse=True)
        wait_clock.add_sem_waits(
            holder.ins, ScopedClock({None: tick_clock.global_clock})
        )
        si = holder.ins.sync_info
        waits = list(si.on_wait) if si else []
        if si:
            si.on_wait[:] = waits[:1]
        for w in waits[1:]:
            n = nc.sync.nop(nofuse=True)
            if n.ins.sync_info is None:
                n.ins.sync_info = mybir.SyncInfo(on_wait=[w], on_update=[])
            else:
                n.ins.sync_info.on_wait.append(w)
        nc.sync.drain()
        nc.all_engine_barrier()
        assert self.sems is not None
        popped = nc._tile_sem_poison_stack.pop()
        assert popped is self._sem_poison
        nc.clear_and_free_semaphores(list(self.sems.allocated().values()))
        nc.all_engine_barrier()

    tile.TileContext._drain_and_barrier = _drain_and_barrier
    tile.TileContext._arma_drain_patched = True


def _split_waits(nc):
    """Cap every instruction at one sync wait.

    This container's walrus rejects instructions with more than one or two
    sync waits ("Too many sync wait commands"). Hoist extra waits onto
    same-engine NOPs inserted immediately before the instruction.
    """
    import concourse.mybir as mybir

    eng = {
        mybir.EngineType.Activation: nc.scalar,
        mybir.EngineType.DVE: nc.vector,
        mybir.EngineType.PE: nc.tensor,
        mybir.EngineType.Pool: nc.gpsimd,
        mybir.EngineType.SP: nc.sync,
    }
    f = nc.m.functions[0]

    def steal_nop(engine):
        nop = eng[engine].nop(nofuse=True).ins
        for bb in f.blocks:
            l = bb.instructions
            if l and l[-1].name == nop.name:
                l.pop()
                break
        return nop

    for b in f.blocks:
        il = b.instructions
        i = 0
        while i < len(il):
            inst = il[i]
            si = inst.sync_info
            if si is not None and len(si.on_wait) > 1:
                extras = list(si.on_wait[1:])
                si.on_wait[:] = si.on_wait[:1]
                for w in extras:
                    nop = steal_nop(inst.engine)
                    nop.sync_info = mybir.SyncInfo(on_wait=[w], on_update=[])
                    il.insert(i, nop)
                    i += 1
            i += 1


def _build_program(T):
    import concourse.bass as bass
    import concourse.mybir as mybir
    import concourse.tile as tile

    _patch_tile_drain()
    dt = mybir.dt
    TT = min(512, T)
    nc = bass.Bass("TRN2", target_bir_lowering=False, debug=False,
                   num_devices=NCORES)

    TP = T + MASZ  # padded time length for x
    xt = nc.declare_dram_parameter("xt", [128, 2 * BL * TP], dt.bfloat16,
                                   isOutput=False)
    wtap = nc.declare_dram_parameter("wtap", [128, ARSZ * 4 * 128],
                                     dt.bfloat16, isOutput=False)
    wma = nc.declare_dram_parameter("wma", [128, 18 * 2 * 128], dt.bfloat16,
                                    isOutput=False)
    barw = nc.declare_dram_parameter("barw", [1, 256], dt.bfloat16,
                                     isOutput=False)
    bma = nc.declare_dram_parameter("bma", [128, 2], dt.float32,
                                    isOutput=False)
    yout = nc.declare_dram_parameter("yout", [128, T * 16], dt.bfloat16,
                                     isOutput=True)

    Tanh = mybir.ActivationFunctionType.Tanh

    with tile.TileContext(nc) as tc:
        with (
            tc.tile_pool(name="w", bufs=1) as wpool,
            tc.tile_pool(name="ybuf", bufs=1) as ypool,
            tc.tile_pool(name="mabuf", bufs=1) as mapool,
            tc.tile_pool(name="xs", bufs=2) as xpool,
            tc.tile_pool(name="sc", bufs=4) as scpool,
            tc.tile_pool(name="pma", bufs=2, space="PSUM") as pmapool,
            tc.tile_pool(name="psc", bufs=2, space="PSUM") as pscpool,
        ):
            wtap_sb = wpool.tile([128, ARSZ * 4 * 128], dt.bfloat16)
            nc.sync.dma_start(out=wtap_sb[:], in_=wtap[:])
            wma_sb = wpool.tile([128, 18 * 2 * 128], dt.bfloat16)
            nc.sync.dma_start(out=wma_sb[:], in_=wma[:])
            barw_sb = wpool.tile([1, 256], dt.bfloat16)
            nc.sync.dma_start(out=barw_sb[:], in_=barw[:])
            bma_sb = wpool.tile([128, 2], dt.float32)
            nc.sync.dma_start(out=bma_sb[:], in_=bma[:])
            ones_sb = wpool.tile([1, 64], dt.bfloat16)
            nc.vector.memset(ones_sb[:], 1.0)

            # y history: col (t+8)*16 + m*8 + b ; first 8 steps are zeros
            y_sb = ypool.tile([128, (T + ARSZ) * 16], dt.bfloat16)
            nc.vector.memset(y_sb[:, 0:ARSZ * 16], 0.0)

            # ma, split per time-tile to keep dep ranges tight
            n_tt = T // TT
            ma_sb = [mapool.tile([128, TT * 16], dt.bfloat16,
                                 name=f"ma{i}", tag=f"ma{i}")
                     for i in range(n_tt)]

            def w_tile(d, k, m):
                i = ((d - 1) * 4 + k * 2 + m) * 128
                return wtap_sb[:, i:i + 128]

            # ---------------- MA phase ----------------
            TTH = TT + MASZ
            for tt in range(n_tt):
                xt_t = xpool.tile([128, 16 * TTH], dt.bfloat16)
                src = xt[:].rearrange("p (s tp) -> p s tp", tp=TP)[
                    :, :, tt * TT:tt * TT + TTH]
                nc.sync.dma_start(
                    out=xt_t[:].rearrange("p (s c) -> p s c", s=16), in_=src)
                for b in range(BL):
                    for m in range(2):
                        ps = pmapool.tile([128, TT], dt.float32)
                        for kk in range(18):
                            w = kk // 2
                            kf = kk % 2
                            c0 = (kf * BL + b) * TTH + w
                            nc.tensor.matmul(
                                ps[:],
                                lhsT=wma_sb[:, (kk * 2 + m) * 128:
                                            (kk * 2 + m + 1) * 128],
                                rhs=xt_t[:, c0:c0 + TT],
                                start=(kk == 0),
                                stop=(kk == 17),
                            )
                        dst = ma_sb[tt][:].rearrange(
                            "p (t c) -> p t c", c=16)[:, :, m * 8 + b:
                                                      m * 8 + b + 1]
                        nc.scalar.activation(
                            dst, ps[:].rearrange("p (t o) -> p t o", o=1),
                            Tanh, bias=bma_sb[:, m:m + 1])

            # ---------------- AR scan ----------------
            y3 = y_sb[:].rearrange("p (t c) -> p t c", c=16)

            def fire(ps, T0, d, S, n, stop=False):
                # tap lag d applied to steps S..S+n-1 (psum block at T0)
                ps3 = ps[:].rearrange("p (i c) -> p i c", c=16)
                i0 = S - T0
                for m in range(2):
                    for k in range(2):
                        dst = ps3[:, i0:i0 + n, m * 8:m * 8 + 8]
                        rhs = y3[:, S - d + ARSZ:S - d + ARSZ + n,
                                 k * 8:k * 8 + 8]
                        nc.tensor.matmul(
                            dst, lhsT=w_tile(d, k, m), rhs=rhs,
                            start=False, stop=stop and k == 1,
                            skip_group_check=True,
                        )

            for blk in range(T // 8):
                T0 = blk * 8
                ps = pscpool.tile([128, 128], dt.float32)
                ps3 = ps[:].rearrange("p (i c) -> p i c", c=16)
                # init with b_ar (K=1 matmul, start=True covers all cols)
                for m in range(2):
                    nc.tensor.matmul(
                        ps3[:, :, m * 8:m * 8 + 8],
                        lhsT=barw_sb[0:1, m * 128:(m + 1) * 128],
                        rhs=ones_sb[0:1, :], start=True, stop=False,
                        skip_group_check=True,
                    )
                for i in range(8):
                    t = T0 + i
                    if i == 0:
                        fire(ps, T0, 8, T0, 8)
                    if i in (0, 4):
                        for d in (4, 5, 6, 7):
                            fire(ps, T0, d, t, 4)
                    if i % 2 == 0:
                        for d in (2, 3):
                            fire(ps, T0, d, t, 2)
                    fire(ps, T0, 1, t, 1, stop=True)
                    ar = scpool.tile([128, 16], dt.bfloat16, tag="ar")
                    nc.scalar.activation(ar[:], ps[:, i * 16:(i + 1) * 16],
                                         Tanh)
                    z = scpool.tile([128, 16], dt.bfloat16, tag="z")
                    nc.vector.tensor_add(
                        z[:], ar[:],
                        ma_sb[t // TT][:, (t % TT) * 16:(t % TT) * 16 + 16])
                    nc.scalar.activation(
                        y_sb[:, (t + ARSZ) * 16:(t + ARSZ + 1) * 16],
                        z[:], Tanh)
                if blk % 8 == 7:
                    c0 = (blk - 7) * 8 * 16
                    nc.sync.dma_start(
                        out=yout[:, c0:c0 + 64 * 16],
                        in_=y_sb[:, c0 + ARSZ * 16:c0 + (64 + ARSZ) * 16])
    _split_waits(nc)
    return nc


def _prep_weights(W_ar, b_ar, W_ma, b_ma):
    # wtap: tap lag d in 1..8, tile (k, m):
    #   wtap[p, ((d-1)*4 + k*2 + m)*128 + q] = W_ar[m*128+q, (8-d)*256 + k*128 + p]
    wt = np.empty((128, ARSZ * 4 * 128), np.float32)
    for d in range(1, 9):
        blkc = W_ar[:, (8 - d) * F:(9 - d) * F]  # [fout, fin]
        for k in range(2):
            for m in range(2):
                tilev = blkc[m * 128:(m + 1) * 128, k * 128:(k + 1) * 128].T
                i = ((d - 1) * 4 + k * 2 + m) * 128
                wt[:, i:i + 128] = tilev
    # wma: tile (kk, m): wma[p, (kk*2+m)*128+q] = W_ma[m*128+q, kk*128+p]
    wm = np.empty((128, 18 * 2 * 128), np.float32)
    WmaT = W_ma.T  # [2304, 256]
    for kk in range(18):
        for m in range(2):
            i = (kk * 2 + m) * 128
            wm[:, i:i + 128] = WmaT[kk * 128:(kk + 1) * 128,
                                    m * 128:(m + 1) * 128]
    bmah = np.empty((128, 2), np.float32)
    bmah[:, 0] = b_ma[:128]
    bmah[:, 1] = b_ma[128:]
    return _bf16(wt), _bf16(wm), _bf16(b_ar.reshape(1, 256)), bmah


def _prep_x(xs, T):
    # xt[p, (kf*BL + b)*(T+8) + tp] = x[b, tp-8, kf*128+p]  (0 for tp<8)
    TP = T + MASZ
    xpad = np.zeros((BL, TP, F), np.float32)
    xpad[:, MASZ:, :] = xs
    # -> [f, b, tp] -> [2, 128, b, tp] -> [128, 2, b, tp]
    a = xpad.transpose(2, 0, 1).reshape(2, 128, BL, TP).transpose(1, 0, 2, 3)
    return _bf16(np.ascontiguousarray(a.reshape(128, 2 * BL * TP)))


def _unshard_y(youts, T):
    # yout[p, t*16 + m*8 + b] = y[b, t, m*128+p]
    out = np.empty((B_TOT, T, F), np.float32)
    for c, yo in enumerate(youts):
        a = np.asarray(yo).astype(np.float32).reshape(128, T, 2, BL)
        out[c * BL:(c + 1) * BL] = a.transpose(3, 1, 2, 0).reshape(BL, T, F)
    return out


def kernel(x, W_ar, b_ar, W_ma, b_ma):
    from concourse.bass_utils import run_bass_kernel_spmd

    T = int(os.environ.get("ARMA_T", T_FULL))
    x = np.asarray(x, np.float32)[:, :T, :]
    W_ar = np.asarray(W_ar, np.float32)
    b_ar = np.asarray(b_ar, np.float32)
    W_ma = np.asarray(W_ma, np.float32)
    b_ma = np.asarray(b_ma, np.float32)

    if T not in _CACHE:
        _CACHE[T] = _build_program(T)
    nc = _CACHE[T]

    wt, wm, barw, bmah = _prep_weights(W_ar, b_ar, W_ma, b_ma)
    in_maps = []
    for c in range(NCORES):
        in_maps.append({
            "xt": _prep_x(x[c * BL:(c + 1) * BL], T),
            "wtap": wt,
            "wma": wm,
            "barw": barw,
            "bma": bmah,
        })
    res = run_bass_kernel_spmd(nc, in_maps, list(range(NCORES)),
                               trace=bool(int(os.environ.get("ARMA_TRACE", "0"))))
    kernel.last_results = res
    return _unshard_y([res.results[c]["yout"] for c in range(NCORES)], T)
